# revision 30
# baseline (speedup 1.0000x reference)
"""Multi-Head Latent Attention (MLA) TRN2 Bass kernel, 8-core parallel. v2.

Sharding: batch x heads. Cores 0-3 own batch 0, cores 4-7 batch 1; within a
batch group each core owns 4 heads (tensor-parallel on q/kv_up/o_proj).
Each core computes the latent projection for its batch (4x replicated),
q/kv projections for its heads, attention, and a partial o_proj; the host
sums the 4 partials per batch and stacks the batches.

All data is bf16 (PE full speed, half the DMA/SBUF of fp32r, and well
within the 2e-2 error budget); PSUM accumulation is fp32.

Dataflow is fully "transposed" so the only on-device transposes are cheap
PE [128,128] block transposes of kv:
  xT [D, S] (host-side transpose, per batch) ->
  latT = Wdown^T xT, qT = Wq^T xT, kvT = Wup^T latT   (all [*, S], SBUF)
  kv_nat[st] = PE-transpose of kvT blocks              ([S-tile, 4*Dh])
  scoresT[keys, q] = kvT^T(block) qT;  expT = exp(scoresT * scale)
  outT[Dh, q]  = kv_nat^T(block) expT  (psum accumulate over key tiles)
  acc[*, q]    = sum_kt expT           (DVE, bf16)
  denom        = ones^T (acc_d + acc_g) (matmul), rcp = 1/denom
  outT_norm    = outT * rcp
  finalT[D, S] = sum_hh (wo_hh block)^T outT_norm[hh]  ([D, S] partial out)
Softmax max-subtraction is skipped: scores are ~N(0, 0.037), |s| < ~1.5.

qT stays in SBUF (no DRAM staging round-trip). Attention runs in 2 query
passes of 1024; o_proj chains for pass 0 drain one-per-2-key-tiles inside
pass 1's loops so their PE/DMA work fills dependency-stall gaps.
"""
import sys

sys.path.insert(0, "/opt/trn_rl_repo")

import numpy as np  # noqa: E402

B = 2
S = 2048
D = 2048
H = 16
DH = 128
DL = 512
P = 128
N_CORES = 8
H_LOC = 4                     # heads per core
HW = H_LOC * DH               # 512
SCALE = float(1.0 / np.sqrt(DH))

D_T = D // P                  # 16
L_T = DL // P                 # 4
S_SL = 512                    # projection s-slice width
N_SL = S // S_SL              # 4
QW = 1024                     # attention query-pass width
N_QP = S // QW                # 2
KT = S // P                   # 16
ST = S // P                   # 16


def _build_nc():
    import concourse.tile as tile
    import concourse.mybir as mybir
    from concourse import bacc

    f32 = mybir.dt.float32
    bf16 = mybir.dt.bfloat16
    f8 = mybir.dt.float8e4
    DR = mybir.MatmulPerfMode.DoubleRow
    EXP = mybir.ActivationFunctionType.Exp
    HP = P // 2

    nc = bacc.Bacc("TRN2", target_bir_lowering=False, debug=False)

    xT = nc.dram_tensor("xT", [D, S], bf16, kind="ExternalInput").ap()
    wq = nc.dram_tensor("wq", [D, HW], bf16, kind="ExternalInput").ap()
    wdown = nc.dram_tensor("wdown", [D, DL], bf16, kind="ExternalInput").ap()
    wup = nc.dram_tensor("wup", [DL, HW], bf16, kind="ExternalInput").ap()
    wo = nc.dram_tensor("wo", [HW, D], bf16, kind="ExternalInput").ap()
    ones_d = nc.dram_tensor("ones", [P, P], bf16, kind="ExternalInput").ap()
    out_d = nc.dram_tensor("outT", [D, S], bf16, kind="ExternalOutput").ap()

    with tile.TileContext(nc) as tc:
        with tc.tile_pool(name="w", bufs=1) as wp, \
             tc.tile_pool(name="xs", bufs=1) as xsp, \
             tc.tile_pool(name="big", bufs=1) as bigp, \
             tc.tile_pool(name="sm", bufs=1) as smp, \
             tc.tile_pool(name="ps", bufs=1, space="PSUM") as psp:

            # ---- weights + first-slice xs, interleaved for fast start ----
            wdown_t = []
            xs0 = []
            for dt_i in range(D_T):
                t = wp.tile([P, DL], bf16, tag=f"wd_{dt_i}", name=f"wd_{dt_i}")
                nc.sync.dma_start(t[:], wdown[dt_i * P:(dt_i + 1) * P, :])
                wdown_t.append(t)
                t = xsp.tile([P, S_SL], bf16, tag=f"xs_{dt_i}", bufs=2,
                             name=f"xs_0_{dt_i}")
                nc.sync.dma_start(t[:], xT[dt_i * P:(dt_i + 1) * P, 0:S_SL])
                xs0.append(t)
            wq_t = []
            for dt_i in range(D_T):
                t = wp.tile([P, HW], bf16, tag=f"wq_{dt_i}", name=f"wq_{dt_i}")
                nc.sync.dma_start(t[:], wq[dt_i * P:(dt_i + 1) * P, :])
                wq_t.append(t)
            ones_t = wp.tile([P, P], bf16, tag="ones", name="ones")
            nc.sync.dma_start(ones_t[:], ones_d[:, :])
            wup_t = []
            for lt in range(L_T):
                t = wp.tile([P, HW], bf16, tag=f"wu_{lt}", name=f"wu_{lt}")
                nc.sync.dma_start(t[:], wup[lt * P:(lt + 1) * P, :])
                wup_t.append(t)
            wo_t = []
            for hh in range(H_LOC):
                t = wp.tile([P, D], bf16, tag=f"wo_{hh}", name=f"wo_{hh}")
                nc.sync.dma_start(t[:], wo[hh * P:(hh + 1) * P, :])
                wo_t.append(t)

            latT = [bigp.tile([P, S], bf16, tag=f"latT_{m}", name=f"latT_{m}")
                    for m in range(L_T)]
            # q and k live as fp8e4 in DoubleRow pair layout: plane i
            # (free dim 1) holds head-dim half i, partitions 0:64 only.
            # Upper psum halves land in scr[*][64:128,:] and are moved into
            # plane 1 by a byte-shuffle DMA once the tensor is complete.
            q8 = [bigp.tile([P, 2, S], f8, tag=f"q8_{m}", name=f"q8_{m}")
                  for m in range(H_LOC)]
            scr = [bigp.tile([P, S], f8, tag=f"scr_{m}", name=f"scr_{m}")
                   for m in range(H_LOC)]

            # ---- Phase A: latent + q projections, streamed over s-slices
            for j in range(N_SL):
                if j == 0:
                    xs = xs0
                else:
                    xs = []
                    for dt_i in range(D_T):
                        t = xsp.tile([P, S_SL], bf16, tag=f"xs_{dt_i}", bufs=2,
                                     name=f"xs_{j}_{dt_i}")
                        nc.sync.dma_start(t[:], xT[dt_i * P:(dt_i + 1) * P,
                                                   j * S_SL:(j + 1) * S_SL])
                        xs.append(t)
                jsl = slice(j * S_SL, (j + 1) * S_SL)
                for m in range(L_T):
                    ps = psp.tile([P, S_SL], f32, tag="pa", bufs=2,
                                  name=f"psA_{j}_l{m}")
                    for dt_i in range(D_T):
                        nc.tensor.matmul(ps[:],
                                         wdown_t[dt_i][:, m * P:(m + 1) * P],
                                         xs[dt_i][:],
                                         start=(dt_i == 0),
                                         stop=(dt_i == D_T - 1))
                    nc.vector.tensor_copy(latT[m][:, jsl], ps[:])
                for m in range(H_LOC):
                    ps = psp.tile([P, S_SL], f32, tag="pa", bufs=2,
                                  name=f"psA_{j}_q{m}")
                    for dt_i in range(D_T):
                        nc.tensor.matmul(ps[:],
                                         wq_t[dt_i][:, m * P:(m + 1) * P],
                                         xs[dt_i][:],
                                         start=(dt_i == 0),
                                         stop=(dt_i == D_T - 1))
                    nc.vector.tensor_copy(q8[m][0:HP, 0, jsl], ps[0:HP, :])
                    nc.vector.tensor_copy(scr[m][HP:P, jsl], ps[HP:P, :])
            for m in range(H_LOC):
                nc.gpsimd.dma_start(q8[m][0:HP, 1, :], scr[m][HP:P, :])

            # ---- Phase A1: kv projections ----
            kv8 = [bigp.tile([P, 2, S], f8, tag=f"kv8_{m}", name=f"kv8_{m}")
                   for m in range(H_LOC)]
            # kv_nat[st]: [128 keys, 512 dh]; reuses the wd_* weight slots
            kvn = [wp.tile([P, HW], bf16, tag=f"wd_{st}", name=f"kvn_{st}")
                   for st in range(ST)]
            outT = [bigp.tile([P, S], bf16, tag=f"outT_{m}", name=f"outT_{m}")
                    for m in range(H_LOC)]

            for hh in range(H_LOC):
                for j in range(N_SL):
                    jsl = slice(j * S_SL, (j + 1) * S_SL)
                    ps = psp.tile([P, S_SL], f32, tag="pa", bufs=2,
                                  name=f"psK_{hh}_{j}")
                    for lt in range(L_T):
                        nc.tensor.matmul(ps[:],
                                         wup_t[lt][:, hh * P:(hh + 1) * P],
                                         latT[lt][:, jsl],
                                         start=(lt == 0), stop=(lt == L_T - 1))
                    nc.vector.tensor_copy(kv8[hh][0:HP, 0, jsl], ps[0:HP, :])
                    nc.vector.tensor_copy(scr[hh][HP:P, jsl], ps[HP:P, :])
                nc.gpsimd.dma_start(kv8[hh][0:HP, 1, :], scr[hh][HP:P, :])
            for st in range(ST):
                ps = psp.tile([P, S_SL], f32, tag="pa", bufs=2,
                              name=f"psN_{st}")
                for lt in range(L_T):
                    nc.tensor.matmul(ps[:],
                                     latT[lt][:, st * P:(st + 1) * P],
                                     wup_t[lt][:],
                                     start=(lt == 0), stop=(lt == L_T - 1))
                nc.vector.tensor_copy(kvn[st][:], ps[:])

            # ---- Phase B: attention (2 query passes) + o_proj drains ----
            # o_proj chain (dc, qc): finalT[dc*128:(dc+1)*128, qc*512:...]
            #   = sum_hh outT[hh]-block^T via psum accumulate; queued after a
            # pass's normalize, drained one-per-2-kt inside later loops.
            pending = []

            def _c_chain(dc, qc):
                pc = psp.tile([P, S_SL], f32, tag="pa", bufs=2,
                              name=f"psC_{dc}_{qc}")
                for hh in range(H_LOC):
                    nc.tensor.matmul(
                        pc[:],
                        wo_t[hh][:, dc * P:(dc + 1) * P],
                        outT[hh][:, qc * S_SL:(qc + 1) * S_SL],
                        start=(hh == 0), stop=(hh == H_LOC - 1))
                fin = smp.tile([P, S_SL], bf16, tag=f"fin_{dc % 4}",
                               bufs=2, name=f"fin_{dc}_{qc}")
                nc.vector.tensor_copy(fin[:], pc[:])
                nc.gpsimd.dma_start(
                    out_d[dc * P:(dc + 1) * P, qc * S_SL:(qc + 1) * S_SL],
                    fin[:])

            def _attn_pass(hh, q0, qw, pop_every):
                nsub = qw // S_SL
                ps_o = psp.tile([P, QW], f32, tag="pso", bufs=1,
                                name=f"pso_{hh}_{q0}")
                acc_d = smp.tile([P, QW], bf16, tag="accd", bufs=2,
                                 name=f"accd_{hh}_{q0}")
                acc_g = smp.tile([P, QW], bf16, tag="accg", bufs=2,
                                 name=f"accg_{hh}_{q0}")
                es = [None] * KT

                def _consume(kt):
                    e = es[kt]
                    for i in range(nsub):
                        nc.tensor.matmul(ps_o[:, i * S_SL:(i + 1) * S_SL],
                                         kvn[kt][:, hh * P:(hh + 1) * P],
                                         e[:, i * S_SL:(i + 1) * S_SL],
                                         start=(kt == 0),
                                         stop=(kt == KT - 1))
                    acc = acc_d if kt % 2 == 0 else acc_g
                    if kt < 2:
                        nc.vector.tensor_copy(acc[:, :qw], e[:, :qw])
                    else:
                        nc.vector.tensor_add(acc[:, :qw], acc[:, :qw],
                                             e[:, :qw])

                for kt in range(KT):
                    ps_s = psp.tile([P, QW], f32, tag="sc", bufs=2,
                                    name=f"pss_{hh}_{q0}_{kt}")
                    for i in range(nsub):
                        nc.tensor.matmul(
                            ps_s[:, i * S_SL:(i + 1) * S_SL],
                            kv8[hh][0:HP, :, kt * P:(kt + 1) * P],
                            q8[hh][0:HP, :,
                                   q0 + i * S_SL:q0 + (i + 1) * S_SL],
                            start=True, stop=True, perf_mode=DR)
                    e = smp.tile([P, QW], bf16, tag="e", bufs=3,
                                 name=f"e_{hh}_{q0}_{kt}")
                    nc.scalar.activation(e[:, :qw], ps_s[:, :qw], EXP,
                                         scale=SCALE)
                    es[kt] = e
                    if kt >= 1:
                        _consume(kt - 1)
                    if kt % pop_every == pop_every - 1 and pending:
                        pending.pop(0)()
                _consume(KT - 1)

                pd = psp.tile([P, QW], f32, tag="sc", bufs=2,
                              name=f"pd_{hh}_{q0}")
                for i in range(nsub):
                    isl = slice(i * S_SL, (i + 1) * S_SL)
                    nc.tensor.matmul(pd[:, isl], ones_t[:], acc_d[:, isl],
                                     start=True, stop=False)
                    nc.tensor.matmul(pd[:, isl], ones_t[:], acc_g[:, isl],
                                     start=False, stop=True)
                rcp = smp.tile([P, QW], f32, tag="rcp", bufs=2,
                               name=f"rcp_{hh}_{q0}")
                nc.vector.reciprocal_approx_fast(out=rcp[:, :qw],
                                                 in_=pd[:, :qw])
                nc.vector.tensor_mul(outT[hh][:, q0:q0 + qw],
                                     ps_o[:, :qw], rcp[:, :qw])

            # pass schedule: the last head's last 1024 queries run as two
            # 512 sub-passes so half the final o_proj chains drain early
            for hh in range(H_LOC):
                _attn_pass(hh, 0, QW, 2)
            for dc in range(D_T):
                for qc in (0, 1):
                    pending.append(lambda dc=dc, qc=qc: _c_chain(dc, qc))
            for hh in range(H_LOC - 1):
                _attn_pass(hh, QW, QW, 2)
            _attn_pass(H_LOC - 1, QW, S_SL, 2)
            for dc in range(D_T):
                pending.append(lambda dc=dc: _c_chain(dc, 2))
            _attn_pass(H_LOC - 1, QW + S_SL, S_SL, 1)
            for dc in range(D_T):
                pending.append(lambda dc=dc: _c_chain(dc, 3))

            # drain remaining o_proj chains
            for ch in pending:
                ch()
            pending = []

    nc.compile()
    return nc


_NC_CACHE = None


def _get_nc():
    global _NC_CACHE
    if _NC_CACHE is None:
        _NC_CACHE = _build_nc()
    return _NC_CACHE


def _run(x, W_q, W_kv_down, W_kv_up, W_o, trace=False):
    import ml_dtypes
    from concourse.bass_utils import run_bass_kernel_spmd

    bf16 = ml_dtypes.bfloat16
    x = np.asarray(x, dtype=np.float32)
    wq_r = np.asarray(W_q, dtype=np.float32).astype(bf16)
    wdown_r = np.asarray(W_kv_down, dtype=np.float32).astype(bf16)
    wup_r = np.asarray(W_kv_up, dtype=np.float32).astype(bf16)
    wo_r = np.asarray(W_o, dtype=np.float32).astype(bf16)
    ones = np.ones((P, P), dtype=bf16)
    xT_b = [np.ascontiguousarray(x[b].T).astype(bf16) for b in range(B)]

    nc = _get_nc()

    in_maps = []
    for c in range(N_CORES):
        bc = c // 4
        hs = slice((c % 4) * HW, (c % 4 + 1) * HW)
        in_maps.append({
            "xT": xT_b[bc],
            "wq": np.ascontiguousarray(wq_r[:, hs]),
            "wdown": wdown_r,
            "wup": np.ascontiguousarray(wup_r[:, hs]),
            "wo": np.ascontiguousarray(wo_r[hs, :]),
            "ones": ones,
        })

    r = run_bass_kernel_spmd(nc, in_maps, list(range(N_CORES)), trace=trace)
    outs = []
    for bc in range(B):
        acc = None
        for i in range(4):
            part = r.results[4 * bc + i]["outT"].astype(np.float64)
            acc = part if acc is None else acc + part
        outs.append(acc.T)
    return np.stack(outs).astype(np.float32), r


def kernel(x, W_q, W_kv_down, W_kv_up, W_o):
    out, _ = _run(x, W_q, W_kv_down, W_kv_up, W_o, trace=False)
    return out


# revision 35
# speedup vs baseline: 1.1070x; 1.1070x over previous
"""Multi-Head Latent Attention (MLA) TRN2 Bass kernel, 8-core parallel. v2.

Sharding: batch x heads. Cores 0-3 own batch 0, cores 4-7 batch 1; within a
batch group each core owns 4 heads (tensor-parallel on q/kv_up/o_proj).
Each core computes the latent projection for its batch (4x replicated),
q/kv projections for its heads, attention, and a partial o_proj; the host
sums the 4 partials per batch and stacks the batches.

All data is bf16 (PE full speed, half the DMA/SBUF of fp32r, and well
within the 2e-2 error budget); PSUM accumulation is fp32.

Dataflow is fully "transposed" so the only on-device transposes are cheap
PE [128,128] block transposes of kv:
  xT [D, S] (host-side transpose, per batch) ->
  latT = Wdown^T xT, qT = Wq^T xT, kvT = Wup^T latT   (all [*, S], SBUF)
  kv_nat[st] = PE-transpose of kvT blocks              ([S-tile, 4*Dh])
  scoresT[keys, q] = kvT^T(block) qT;  expT = exp(scoresT * scale)
  outT[Dh, q]  = kv_nat^T(block) expT  (psum accumulate over key tiles)
  acc[*, q]    = sum_kt expT           (DVE, bf16)
  denom        = ones^T (acc_d + acc_g) (matmul), rcp = 1/denom
  outT_norm    = outT * rcp
  finalT[D, S] = sum_hh (wo_hh block)^T outT_norm[hh]  ([D, S] partial out)
Softmax max-subtraction is skipped: scores are ~N(0, 0.037), |s| < ~1.5.

qT stays in SBUF (no DRAM staging round-trip). Attention runs in 2 query
passes of 1024; o_proj chains for pass 0 drain one-per-2-key-tiles inside
pass 1's loops so their PE/DMA work fills dependency-stall gaps.
"""
import sys

sys.path.insert(0, "/opt/trn_rl_repo")

import numpy as np  # noqa: E402

B = 2
S = 2048
D = 2048
H = 16
DH = 128
DL = 512
P = 128
N_CORES = 8
H_LOC = 4                     # heads per core
HW = H_LOC * DH               # 512
SCALE = float(1.0 / np.sqrt(DH))

D_T = D // P                  # 16
L_T = DL // P                 # 4
S_SL = 512                    # projection s-slice width
N_SL = S // S_SL              # 4
QW = 1024                     # attention query-pass width
N_QP = S // QW                # 2
KT = S // P                   # 16
ST = S // P                   # 16


def _build_nc():
    import concourse.tile as tile
    import concourse.mybir as mybir
    from concourse import bacc

    f32 = mybir.dt.float32
    bf16 = mybir.dt.bfloat16
    EXP = mybir.ActivationFunctionType.Exp

    nc = bacc.Bacc("TRN2", target_bir_lowering=False, debug=False)

    xT = nc.dram_tensor("xT", [D, S], bf16, kind="ExternalInput").ap()
    wq = nc.dram_tensor("wq", [D, HW], bf16, kind="ExternalInput").ap()
    wdown = nc.dram_tensor("wdown", [D, DL], bf16, kind="ExternalInput").ap()
    wup = nc.dram_tensor("wup", [DL, HW], bf16, kind="ExternalInput").ap()
    wo = nc.dram_tensor("wo", [HW, D], bf16, kind="ExternalInput").ap()
    ones_d = nc.dram_tensor("ones", [P, P], bf16, kind="ExternalInput").ap()
    out_d = nc.dram_tensor("outT", [D, S], bf16, kind="ExternalOutput").ap()

    with tile.TileContext(nc) as tc:
        with tc.tile_pool(name="w", bufs=1) as wp, \
             tc.tile_pool(name="xs", bufs=1) as xsp, \
             tc.tile_pool(name="big", bufs=1) as bigp, \
             tc.tile_pool(name="sm", bufs=1) as smp, \
             tc.tile_pool(name="ps", bufs=1, space="PSUM") as psp:

            # ---- weights + first-slice xs, interleaved for fast start ----
            wdown_t = []
            xs0 = []
            for dt_i in range(D_T):
                t = wp.tile([P, DL], bf16, tag=f"wd_{dt_i}", name=f"wd_{dt_i}")
                nc.sync.dma_start(t[:], wdown[dt_i * P:(dt_i + 1) * P, :])
                wdown_t.append(t)
                t = xsp.tile([P, S_SL], bf16, tag=f"xs_{dt_i}", bufs=2,
                             name=f"xs_0_{dt_i}")
                nc.sync.dma_start(t[:], xT[dt_i * P:(dt_i + 1) * P, 0:S_SL])
                xs0.append(t)
            wq_t = []
            for dt_i in range(D_T):
                t = wp.tile([P, HW], bf16, tag=f"wq_{dt_i}", name=f"wq_{dt_i}")
                nc.sync.dma_start(t[:], wq[dt_i * P:(dt_i + 1) * P, :])
                wq_t.append(t)
            ones_t = wp.tile([P, P], bf16, tag="ones", name="ones")
            nc.sync.dma_start(ones_t[:], ones_d[:, :])
            wup_t = []
            for lt in range(L_T):
                t = wp.tile([P, HW], bf16, tag=f"wu_{lt}", name=f"wu_{lt}")
                nc.sync.dma_start(t[:], wup[lt * P:(lt + 1) * P, :])
                wup_t.append(t)
            wo_t = []
            for hh in range(H_LOC):
                t = wp.tile([P, D], bf16, tag=f"wo_{hh}", name=f"wo_{hh}")
                nc.sync.dma_start(t[:], wo[hh * P:(hh + 1) * P, :])
                wo_t.append(t)

            latT = [bigp.tile([P, S], bf16, tag=f"latT_{m}", name=f"latT_{m}")
                    for m in range(L_T)]
            qT = [bigp.tile([P, S], bf16, tag=f"qT_{m}", name=f"qT_{m}")
                  for m in range(H_LOC)]

            # ---- Phase A: latent + q projections, streamed over s-slices
            for j in range(N_SL):
                if j == 0:
                    xs = xs0
                else:
                    xs = []
                    for dt_i in range(D_T):
                        t = xsp.tile([P, S_SL], bf16, tag=f"xs_{dt_i}", bufs=2,
                                     name=f"xs_{j}_{dt_i}")
                        nc.sync.dma_start(t[:], xT[dt_i * P:(dt_i + 1) * P,
                                                   j * S_SL:(j + 1) * S_SL])
                        xs.append(t)
                jsl = slice(j * S_SL, (j + 1) * S_SL)
                for m in range(L_T):
                    ps = psp.tile([P, S_SL], f32, tag="pa", bufs=2,
                                  name=f"psA_{j}_l{m}")
                    for dt_i in range(D_T):
                        nc.tensor.matmul(ps[:],
                                         wdown_t[dt_i][:, m * P:(m + 1) * P],
                                         xs[dt_i][:],
                                         start=(dt_i == 0),
                                         stop=(dt_i == D_T - 1))
                    nc.vector.tensor_copy(latT[m][:, jsl], ps[:])
                for m in range(H_LOC):
                    ps = psp.tile([P, S_SL], f32, tag="pa", bufs=2,
                                  name=f"psA_{j}_q{m}")
                    for dt_i in range(D_T):
                        nc.tensor.matmul(ps[:],
                                         wq_t[dt_i][:, m * P:(m + 1) * P],
                                         xs[dt_i][:],
                                         start=(dt_i == 0),
                                         stop=(dt_i == D_T - 1))
                    nc.vector.tensor_copy(qT[m][:, jsl], ps[:])

            # ---- Phase A1: kv projections ----
            kvT = [bigp.tile([P, S], bf16, tag=f"kvT_{m}", name=f"kvT_{m}")
                   for m in range(H_LOC)]
            # kv_nat[st]: [128 keys, 512 dh]; reuses the wd_* weight slots
            kvn = [wp.tile([P, HW], bf16, tag=f"wd_{st}", name=f"kvn_{st}")
                   for st in range(ST)]
            outT = [bigp.tile([P, S], bf16, tag=f"outT_{m}", name=f"outT_{m}")
                    for m in range(H_LOC)]

            def _kvt_chain(hh, j):
                jsl = slice(j * S_SL, (j + 1) * S_SL)
                ps = psp.tile([P, S_SL], f32, tag="pa", bufs=2,
                              name=f"psK_{hh}_{j}")
                for lt in range(L_T):
                    nc.tensor.matmul(ps[:],
                                     wup_t[lt][:, hh * P:(hh + 1) * P],
                                     latT[lt][:, jsl],
                                     start=(lt == 0), stop=(lt == L_T - 1))
                nc.vector.tensor_copy(kvT[hh][:, jsl], ps[:])

            def _kvn_chain(st):
                ps = psp.tile([P, S_SL], f32, tag="pa", bufs=2,
                              name=f"psN_{st}")
                for lt in range(L_T):
                    nc.tensor.matmul(ps[:],
                                     latT[lt][:, st * P:(st + 1) * P],
                                     wup_t[lt][:],
                                     start=(lt == 0), stop=(lt == L_T - 1))
                nc.vector.tensor_copy(kvn[st][:], ps[:])

            # only head 0's kvT is needed before attention starts; the rest
            # of the kv work fills the ACT-bound first-query-pass kt loops
            for j in range(N_SL):
                _kvt_chain(0, j)

            # ---- Phase B: attention (2 query passes) + o_proj drains ----
            # o_proj chain (dc, qc): finalT[dc*128:(dc+1)*128, qc*512:...]
            #   = sum_hh outT[hh]-block^T via psum accumulate; queued after a
            # pass's normalize, drained one-per-2-kt inside later loops.
            pending = []

            def _c_chain(dc, qc):
                pc = psp.tile([P, S_SL], f32, tag="pa", bufs=2,
                              name=f"psC_{dc}_{qc}")
                for hh in range(H_LOC):
                    nc.tensor.matmul(
                        pc[:],
                        wo_t[hh][:, dc * P:(dc + 1) * P],
                        outT[hh][:, qc * S_SL:(qc + 1) * S_SL],
                        start=(hh == 0), stop=(hh == H_LOC - 1))
                fin = smp.tile([P, S_SL], bf16, tag=f"fin_{dc % 4}",
                               bufs=2, name=f"fin_{dc}_{qc}")
                nc.vector.tensor_copy(fin[:], pc[:])
                nc.gpsimd.dma_start(
                    out_d[dc * P:(dc + 1) * P, qc * S_SL:(qc + 1) * S_SL],
                    fin[:])

            def _attn_pass(hh, q0, qw, pop_every, fillers=None):
                nsub = qw // S_SL
                ps_o = psp.tile([P, QW], f32, tag="pso", bufs=1,
                                name=f"pso_{hh}_{q0}")
                acc_d = smp.tile([P, QW], bf16, tag="accd", bufs=2,
                                 name=f"accd_{hh}_{q0}")
                acc_g = smp.tile([P, QW], bf16, tag="accg", bufs=2,
                                 name=f"accg_{hh}_{q0}")
                es = [None] * KT

                def _consume(kt):
                    e = es[kt]
                    for i in range(nsub):
                        nc.tensor.matmul(ps_o[:, i * S_SL:(i + 1) * S_SL],
                                         kvn[kt][:, hh * P:(hh + 1) * P],
                                         e[:, i * S_SL:(i + 1) * S_SL],
                                         start=(kt == 0),
                                         stop=(kt == KT - 1))
                    acc = acc_d if kt % 2 == 0 else acc_g
                    if kt < 2:
                        nc.vector.tensor_copy(acc[:, :qw], e[:, :qw])
                    else:
                        nc.vector.tensor_add(acc[:, :qw], acc[:, :qw],
                                             e[:, :qw])

                for kt in range(KT):
                    if fillers is not None:
                        for ch in fillers.get(kt, ()):
                            ch()
                    ps_s = psp.tile([P, QW], f32, tag="sc", bufs=2,
                                    name=f"pss_{hh}_{q0}_{kt}")
                    for i in range(nsub):
                        nc.tensor.matmul(
                            ps_s[:, i * S_SL:(i + 1) * S_SL],
                            kvT[hh][:, kt * P:(kt + 1) * P],
                            qT[hh][:, q0 + i * S_SL:q0 + (i + 1) * S_SL],
                            start=True, stop=True)
                    e = smp.tile([P, QW], bf16, tag="e", bufs=4,
                                 name=f"e_{hh}_{q0}_{kt}")
                    nc.scalar.activation(e[:, :qw], ps_s[:, :qw], EXP,
                                         scale=SCALE)
                    es[kt] = e
                    if kt >= 1:
                        _consume(kt - 1)
                    if kt % pop_every == pop_every - 1 and pending:
                        pending.pop(0)()
                _consume(KT - 1)

                # fold the two exp accumulators, then one ones-matmul chain
                nc.vector.tensor_add(acc_d[:, :qw], acc_d[:, :qw],
                                     acc_g[:, :qw])
                pd = psp.tile([P, QW], f32, tag="sc", bufs=2,
                              name=f"pd_{hh}_{q0}")
                for i in range(nsub):
                    isl = slice(i * S_SL, (i + 1) * S_SL)
                    nc.tensor.matmul(pd[:, isl], ones_t[:], acc_d[:, isl],
                                     start=True, stop=True)
                rcp = smp.tile([P, QW], f32, tag="rcp", bufs=2,
                               name=f"rcp_{hh}_{q0}")
                nc.vector.reciprocal_approx_fast(out=rcp[:, :qw],
                                                 in_=pd[:, :qw])
                nc.vector.tensor_mul(outT[hh][:, q0:q0 + qw],
                                     ps_o[:, :qw], rcp[:, :qw])

            # pass schedule: the last head's last 1024 queries run as two
            # 512 sub-passes so half the final o_proj chains drain early.
            # The first-query-pass loops are ACT(exp)-bound, so the kvn
            # chains and heads 1-3's kvT chains run as fillers inside them.
            f0 = {kt: [lambda kt=kt: _kvn_chain(kt)] for kt in range(KT)}
            for j in range(N_SL):
                f0[4 * j + 3].append(lambda j=j: _kvt_chain(1, j))
            f1 = {4 * j + 3: [lambda j=j: _kvt_chain(2, j)]
                  for j in range(N_SL)}
            f2 = {4 * j + 3: [lambda j=j: _kvt_chain(3, j)]
                  for j in range(N_SL)}
            _attn_pass(0, 0, QW, 2, fillers=f0)
            _attn_pass(1, 0, QW, 2, fillers=f1)
            _attn_pass(2, 0, QW, 2, fillers=f2)
            _attn_pass(3, 0, QW, 2)
            for dc in range(D_T):
                for qc in (0, 1):
                    pending.append(lambda dc=dc, qc=qc: _c_chain(dc, qc))
            for hh in range(H_LOC - 1):
                _attn_pass(hh, QW, QW, 2)
            _attn_pass(H_LOC - 1, QW, S_SL, 2)
            for dc in range(D_T):
                pending.append(lambda dc=dc: _c_chain(dc, 2))
            _attn_pass(H_LOC - 1, QW + S_SL, S_SL, 1)
            for dc in range(D_T):
                pending.append(lambda dc=dc: _c_chain(dc, 3))

            # drain remaining o_proj chains
            for ch in pending:
                ch()
            pending = []

    nc.compile()
    return nc


_NC_CACHE = None


def _get_nc():
    global _NC_CACHE
    if _NC_CACHE is None:
        _NC_CACHE = _build_nc()
    return _NC_CACHE


def _run(x, W_q, W_kv_down, W_kv_up, W_o, trace=False):
    import ml_dtypes
    from concourse.bass_utils import run_bass_kernel_spmd

    bf16 = ml_dtypes.bfloat16
    x = np.asarray(x, dtype=np.float32)
    wq_r = np.asarray(W_q, dtype=np.float32).astype(bf16)
    wdown_r = np.asarray(W_kv_down, dtype=np.float32).astype(bf16)
    wup_r = np.asarray(W_kv_up, dtype=np.float32).astype(bf16)
    wo_r = np.asarray(W_o, dtype=np.float32).astype(bf16)
    ones = np.ones((P, P), dtype=bf16)
    xT_b = [np.ascontiguousarray(x[b].T).astype(bf16) for b in range(B)]

    nc = _get_nc()

    in_maps = []
    for c in range(N_CORES):
        bc = c // 4
        hs = slice((c % 4) * HW, (c % 4 + 1) * HW)
        in_maps.append({
            "xT": xT_b[bc],
            "wq": np.ascontiguousarray(wq_r[:, hs]),
            "wdown": wdown_r,
            "wup": np.ascontiguousarray(wup_r[:, hs]),
            "wo": np.ascontiguousarray(wo_r[hs, :]),
            "ones": ones,
        })

    r = run_bass_kernel_spmd(nc, in_maps, list(range(N_CORES)), trace=trace)
    outs = []
    for bc in range(B):
        acc = None
        for i in range(4):
            part = r.results[4 * bc + i]["outT"].astype(np.float64)
            acc = part if acc is None else acc + part
        outs.append(acc.T)
    return np.stack(outs).astype(np.float32), r


def kernel(x, W_q, W_kv_down, W_kv_up, W_o):
    out, _ = _run(x, W_q, W_kv_down, W_kv_up, W_o, trace=False)
    return out


# revision 38
# speedup vs baseline: 1.1173x; 1.0093x over previous
"""Multi-Head Latent Attention (MLA) TRN2 Bass kernel, 8-core parallel. v2.

Sharding: batch x heads. Cores 0-3 own batch 0, cores 4-7 batch 1; within a
batch group each core owns 4 heads (tensor-parallel on q/kv_up/o_proj).
Each core computes the latent projection for its batch (4x replicated),
q/kv projections for its heads, attention, and a partial o_proj; the host
sums the 4 partials per batch and stacks the batches.

All data is bf16 (PE full speed, half the DMA/SBUF of fp32r, and well
within the 2e-2 error budget); PSUM accumulation is fp32.

Dataflow is fully "transposed" so the only on-device transposes are cheap
PE [128,128] block transposes of kv:
  xT [D, S] (host-side transpose, per batch) ->
  latT = Wdown^T xT, qT = Wq^T xT, kvT = Wup^T latT   (all [*, S], SBUF)
  kv_nat[st] = PE-transpose of kvT blocks              ([S-tile, 4*Dh])
  scoresT[keys, q] = kvT^T(block) qT;  expT = exp(scoresT * scale)
  outT[Dh, q]  = kv_nat^T(block) expT  (psum accumulate over key tiles)
  acc[*, q]    = sum_kt expT           (DVE, bf16)
  denom        = ones^T (acc_d + acc_g) (matmul), rcp = 1/denom
  outT_norm    = outT * rcp
  finalT[D, S] = sum_hh (wo_hh block)^T outT_norm[hh]  ([D, S] partial out)
Softmax max-subtraction is skipped: scores are ~N(0, 0.037), |s| < ~1.5.

qT stays in SBUF (no DRAM staging round-trip). Attention runs in 2 query
passes of 1024; o_proj chains for pass 0 drain one-per-2-key-tiles inside
pass 1's loops so their PE/DMA work fills dependency-stall gaps.
"""
import sys

sys.path.insert(0, "/opt/trn_rl_repo")

import numpy as np  # noqa: E402

B = 2
S = 2048
D = 2048
H = 16
DH = 128
DL = 512
P = 128
N_CORES = 8
H_LOC = 4                     # heads per core
HW = H_LOC * DH               # 512
SCALE = float(1.0 / np.sqrt(DH))

D_T = D // P                  # 16
L_T = DL // P                 # 4
S_SL = 512                    # projection s-slice width
N_SL = S // S_SL              # 4
QW = 1024                     # attention query-pass width
N_QP = S // QW                # 2
KT = S // P                   # 16
ST = S // P                   # 16


def _build_nc():
    import concourse.tile as tile
    import concourse.mybir as mybir
    from concourse import bacc

    f32 = mybir.dt.float32
    bf16 = mybir.dt.bfloat16
    EXP = mybir.ActivationFunctionType.Exp

    nc = bacc.Bacc("TRN2", target_bir_lowering=False, debug=False)

    xT = nc.dram_tensor("xT", [D, S], bf16, kind="ExternalInput").ap()
    wq = nc.dram_tensor("wq", [D, HW], bf16, kind="ExternalInput").ap()
    wdown = nc.dram_tensor("wdown", [D, DL], bf16, kind="ExternalInput").ap()
    wup = nc.dram_tensor("wup", [DL, HW], bf16, kind="ExternalInput").ap()
    wo = nc.dram_tensor("wo", [HW, D], bf16, kind="ExternalInput").ap()
    ones_d = nc.dram_tensor("ones", [P, P], bf16, kind="ExternalInput").ap()
    out_d = nc.dram_tensor("outT", [D, S], bf16, kind="ExternalOutput").ap()

    with tile.TileContext(nc) as tc:
        with tc.tile_pool(name="w", bufs=1) as wp, \
             tc.tile_pool(name="xs", bufs=1) as xsp, \
             tc.tile_pool(name="big", bufs=1) as bigp, \
             tc.tile_pool(name="sm", bufs=1) as smp, \
             tc.tile_pool(name="ps", bufs=1, space="PSUM") as psp:

            # ---- weights + first-slice xs, interleaved for fast start ----
            wdown_t = []
            xs0 = []
            for dt_i in range(D_T):
                t = wp.tile([P, DL], bf16, tag=f"wd_{dt_i}", name=f"wd_{dt_i}")
                nc.sync.dma_start(t[:], wdown[dt_i * P:(dt_i + 1) * P, :])
                wdown_t.append(t)
                t = xsp.tile([P, S_SL], bf16, tag=f"xs_{dt_i}", bufs=2,
                             name=f"xs_0_{dt_i}")
                nc.sync.dma_start(t[:], xT[dt_i * P:(dt_i + 1) * P, 0:S_SL])
                xs0.append(t)
            wq_t = []
            for dt_i in range(D_T):
                t = wp.tile([P, HW], bf16, tag=f"wq_{dt_i}", name=f"wq_{dt_i}")
                nc.sync.dma_start(t[:], wq[dt_i * P:(dt_i + 1) * P, :])
                wq_t.append(t)
            ones_t = wp.tile([P, P], bf16, tag="ones", name="ones")
            nc.sync.dma_start(ones_t[:], ones_d[:, :])
            wup_t = []
            for lt in range(L_T):
                t = wp.tile([P, HW], bf16, tag=f"wu_{lt}", name=f"wu_{lt}")
                nc.sync.dma_start(t[:], wup[lt * P:(lt + 1) * P, :])
                wup_t.append(t)
            wo_t = []
            for hh in range(H_LOC):
                t = wp.tile([P, D], bf16, tag=f"wo_{hh}", name=f"wo_{hh}")
                nc.sync.dma_start(t[:], wo[hh * P:(hh + 1) * P, :])
                wo_t.append(t)

            latT = [bigp.tile([P, S], bf16, tag=f"latT_{m}", name=f"latT_{m}")
                    for m in range(L_T)]
            qT = [bigp.tile([P, S], bf16, tag=f"qT_{m}", name=f"qT_{m}")
                  for m in range(H_LOC)]

            # ---- Phase A: latent + q projections, streamed over s-slices
            for j in range(N_SL):
                if j == 0:
                    xs = xs0
                else:
                    xs = []
                    for dt_i in range(D_T):
                        t = xsp.tile([P, S_SL], bf16, tag=f"xs_{dt_i}", bufs=2,
                                     name=f"xs_{j}_{dt_i}")
                        nc.sync.dma_start(t[:], xT[dt_i * P:(dt_i + 1) * P,
                                                   j * S_SL:(j + 1) * S_SL])
                        xs.append(t)
                jsl = slice(j * S_SL, (j + 1) * S_SL)
                for m in range(L_T):
                    ps = psp.tile([P, S_SL], f32, tag="pa", bufs=2,
                                  name=f"psA_{j}_l{m}")
                    for dt_i in range(D_T):
                        nc.tensor.matmul(ps[:],
                                         wdown_t[dt_i][:, m * P:(m + 1) * P],
                                         xs[dt_i][:],
                                         start=(dt_i == 0),
                                         stop=(dt_i == D_T - 1))
                    nc.vector.tensor_copy(latT[m][:, jsl], ps[:])
                for m in range(H_LOC):
                    ps = psp.tile([P, S_SL], f32, tag="pa", bufs=2,
                                  name=f"psA_{j}_q{m}")
                    for dt_i in range(D_T):
                        nc.tensor.matmul(ps[:],
                                         wq_t[dt_i][:, m * P:(m + 1) * P],
                                         xs[dt_i][:],
                                         start=(dt_i == 0),
                                         stop=(dt_i == D_T - 1))
                    nc.vector.tensor_copy(qT[m][:, jsl], ps[:])

            # ---- Phase A1: kv projections ----
            kvT = [bigp.tile([P, S], bf16, tag=f"kvT_{m}", name=f"kvT_{m}")
                   for m in range(H_LOC)]
            # kv_nat[st]: [128 keys, 512 dh]; reuses the wd_* weight slots
            kvn = [wp.tile([P, HW], bf16, tag=f"wd_{st}", name=f"kvn_{st}")
                   for st in range(ST)]
            outT = [bigp.tile([P, S], bf16, tag=f"outT_{m}", name=f"outT_{m}")
                    for m in range(H_LOC)]

            def _kvt_chain(hh, j):
                jsl = slice(j * S_SL, (j + 1) * S_SL)
                ps = psp.tile([P, S_SL], f32, tag="pa", bufs=2,
                              name=f"psK_{hh}_{j}")
                for lt in range(L_T):
                    nc.tensor.matmul(ps[:],
                                     wup_t[lt][:, hh * P:(hh + 1) * P],
                                     latT[lt][:, jsl],
                                     start=(lt == 0), stop=(lt == L_T - 1))
                nc.vector.tensor_copy(kvT[hh][:, jsl], ps[:])

            def _kvn_chain(st):
                ps = psp.tile([P, S_SL], f32, tag="pa", bufs=2,
                              name=f"psN_{st}")
                for lt in range(L_T):
                    nc.tensor.matmul(ps[:],
                                     latT[lt][:, st * P:(st + 1) * P],
                                     wup_t[lt][:],
                                     start=(lt == 0), stop=(lt == L_T - 1))
                nc.vector.tensor_copy(kvn[st][:], ps[:])

            # only head 0's kvT is needed before attention starts; the rest
            # of the kv work fills the ACT-bound first-query-pass kt loops
            for j in range(N_SL):
                _kvt_chain(0, j)

            # ---- Phase B: attention (2 query passes) + o_proj drains ----
            # o_proj chain (dc, qc): finalT[dc*128:(dc+1)*128, qc*512:...]
            #   = sum_hh outT[hh]-block^T via psum accumulate; queued after a
            # pass's normalize, drained one-per-2-kt inside later loops.
            pending = []

            def _c_chain(dc, qc):
                pc = psp.tile([P, S_SL], f32, tag="pa", bufs=2,
                              name=f"psC_{dc}_{qc}")
                for hh in range(H_LOC):
                    nc.tensor.matmul(
                        pc[:],
                        wo_t[hh][:, dc * P:(dc + 1) * P],
                        outT[hh][:, qc * S_SL:(qc + 1) * S_SL],
                        start=(hh == 0), stop=(hh == H_LOC - 1))
                fin = smp.tile([P, S_SL], bf16, tag=f"fin_{dc % 4}",
                               bufs=2, name=f"fin_{dc}_{qc}")
                nc.vector.tensor_copy(fin[:], pc[:])
                nc.gpsimd.dma_start(
                    out_d[dc * P:(dc + 1) * P, qc * S_SL:(qc + 1) * S_SL],
                    fin[:])

            def _attn_pass(hh, q0, qw, pop_every, fillers=None,
                           prev_epi=None):
                nsub = qw // S_SL
                ps_o = psp.tile([P, QW], f32, tag="pso", bufs=1,
                                name=f"pso_{hh}_{q0}")
                acc_d = smp.tile([P, QW], bf16, tag="accd", bufs=2,
                                 name=f"accd_{hh}_{q0}")
                acc_g = smp.tile([P, QW], bf16, tag="accg", bufs=2,
                                 name=f"accg_{hh}_{q0}")
                es = [None] * KT

                def _consume(kt):
                    e = es[kt]
                    for i in range(nsub):
                        nc.tensor.matmul(ps_o[:, i * S_SL:(i + 1) * S_SL],
                                         kvn[kt][:, hh * P:(hh + 1) * P],
                                         e[:, i * S_SL:(i + 1) * S_SL],
                                         start=(kt == 0),
                                         stop=(kt == KT - 1))
                    acc = acc_d if kt % 2 == 0 else acc_g
                    if kt < 2:
                        nc.vector.tensor_copy(acc[:, :qw], e[:, :qw])
                    else:
                        nc.vector.tensor_add(acc[:, :qw], acc[:, :qw],
                                             e[:, :qw])

                for kt in range(KT):
                    if fillers is not None:
                        for ch in fillers.get(kt, ()):
                            ch()
                    ps_s = psp.tile([P, QW], f32, tag="sc", bufs=2,
                                    name=f"pss_{hh}_{q0}_{kt}")
                    for i in range(nsub):
                        nc.tensor.matmul(
                            ps_s[:, i * S_SL:(i + 1) * S_SL],
                            kvT[hh][:, kt * P:(kt + 1) * P],
                            qT[hh][:, q0 + i * S_SL:q0 + (i + 1) * S_SL],
                            start=True, stop=True)
                    e = smp.tile([P, QW], bf16, tag="e", bufs=4,
                                 name=f"e_{hh}_{q0}_{kt}")
                    nc.scalar.activation(e[:, :qw], ps_s[:, :qw], EXP,
                                         scale=SCALE)
                    es[kt] = e
                    if kt == 0 and prev_epi is not None:
                        # previous pass's denominator/normalize runs here,
                        # behind this pass's first exp, so ACT never waits
                        # for it at the pass boundary (pso has one buf, so
                        # it must finish before this pass's first consume)
                        prev_epi()
                    if kt >= 1:
                        _consume(kt - 1)
                    if kt % pop_every == pop_every - 1 and pending:
                        pending.pop(0)()
                _consume(KT - 1)

                def _epilogue():
                    # fold the exp accumulators, then one ones-matmul chain
                    nc.vector.tensor_add(acc_d[:, :qw], acc_d[:, :qw],
                                         acc_g[:, :qw])
                    pd = psp.tile([P, QW], f32, tag="sc", bufs=2,
                                  name=f"pd_{hh}_{q0}")
                    for i in range(nsub):
                        isl = slice(i * S_SL, (i + 1) * S_SL)
                        nc.tensor.matmul(pd[:, isl], ones_t[:],
                                         acc_d[:, isl],
                                         start=True, stop=True)
                    rcp = smp.tile([P, QW], f32, tag="rcp", bufs=2,
                                   name=f"rcp_{hh}_{q0}")
                    nc.vector.reciprocal_approx_fast(out=rcp[:, :qw],
                                                     in_=pd[:, :qw])
                    nc.vector.tensor_mul(outT[hh][:, q0:q0 + qw],
                                         ps_o[:, :qw], rcp[:, :qw])
                return _epilogue

            # pass schedule: the last head's last 1024 queries run as two
            # 512 sub-passes so half the final o_proj chains drain early.
            # The first-query-pass loops are ACT(exp)-bound, so the kvn
            # chains and heads 1-3's kvT chains run as fillers inside them.
            f0 = {kt: [lambda kt=kt: _kvn_chain(kt)] for kt in range(KT)}
            for j in range(N_SL):
                f0[4 * j + 3].append(lambda j=j: _kvt_chain(1, j))
            f1 = {4 * j + 3: [lambda j=j: _kvt_chain(2, j)]
                  for j in range(N_SL)}
            f2 = {4 * j + 3: [lambda j=j: _kvt_chain(3, j)]
                  for j in range(N_SL)}
            epi = _attn_pass(0, 0, QW, 2, fillers=f0)
            epi = _attn_pass(1, 0, QW, 2, fillers=f1, prev_epi=epi)
            epi = _attn_pass(2, 0, QW, 2, fillers=f2, prev_epi=epi)
            epi = _attn_pass(3, 0, QW, 2, prev_epi=epi)
            for dc in range(D_T):
                for qc in (0, 1):
                    pending.append(lambda dc=dc, qc=qc: _c_chain(dc, qc))
            for hh in range(H_LOC - 1):
                epi = _attn_pass(hh, QW, QW, 2, prev_epi=epi)
            epi = _attn_pass(H_LOC - 1, QW, S_SL, 2, prev_epi=epi)
            for dc in range(D_T):
                pending.append(lambda dc=dc: _c_chain(dc, 2))
            epi = _attn_pass(H_LOC - 1, QW + S_SL, S_SL, 1, prev_epi=epi)
            epi()
            for dc in range(D_T):
                pending.append(lambda dc=dc: _c_chain(dc, 3))

            # drain remaining o_proj chains
            for ch in pending:
                ch()
            pending = []

    nc.compile()
    return nc


_NC_CACHE = None


def _get_nc():
    global _NC_CACHE
    if _NC_CACHE is None:
        _NC_CACHE = _build_nc()
    return _NC_CACHE


def _run(x, W_q, W_kv_down, W_kv_up, W_o, trace=False):
    import ml_dtypes
    from concourse.bass_utils import run_bass_kernel_spmd

    bf16 = ml_dtypes.bfloat16
    x = np.asarray(x, dtype=np.float32)
    wq_r = np.asarray(W_q, dtype=np.float32).astype(bf16)
    wdown_r = np.asarray(W_kv_down, dtype=np.float32).astype(bf16)
    wup_r = np.asarray(W_kv_up, dtype=np.float32).astype(bf16)
    wo_r = np.asarray(W_o, dtype=np.float32).astype(bf16)
    ones = np.ones((P, P), dtype=bf16)
    xT_b = [np.ascontiguousarray(x[b].T).astype(bf16) for b in range(B)]

    nc = _get_nc()

    in_maps = []
    for c in range(N_CORES):
        bc = c // 4
        hs = slice((c % 4) * HW, (c % 4 + 1) * HW)
        in_maps.append({
            "xT": xT_b[bc],
            "wq": np.ascontiguousarray(wq_r[:, hs]),
            "wdown": wdown_r,
            "wup": np.ascontiguousarray(wup_r[:, hs]),
            "wo": np.ascontiguousarray(wo_r[hs, :]),
            "ones": ones,
        })

    r = run_bass_kernel_spmd(nc, in_maps, list(range(N_CORES)), trace=trace)
    outs = []
    for bc in range(B):
        acc = None
        for i in range(4):
            part = r.results[4 * bc + i]["outT"].astype(np.float64)
            acc = part if acc is None else acc + part
        outs.append(acc.T)
    return np.stack(outs).astype(np.float32), r


def kernel(x, W_q, W_kv_down, W_kv_up, W_o):
    out, _ = _run(x, W_q, W_kv_down, W_kv_up, W_o, trace=False)
    return out
